# revision 1
# baseline (speedup 1.0000x reference)
"""Trainium2 Bass kernel for nn_Attn: softmax(enc @ (W^T h)) over seq_len.

Math: energy = enc @ W^T + b; attn = energy @ h; out = softmax(attn).
Algebraically attn[s] = enc[s,:] . v + (b.h) with v = W^T h, and the (b.h)
term is constant across s so softmax cancels it. The device work is the
memory-bound part: streaming the 128 MiB encoder_outputs once, sharded
along seq_len across 8 NeuronCores. Per 128-row block: VectorE multiplies
by v (tensor_tensor), ScalarE reduces rows (activation Copy + accum_out),
so the two passes over the data run on different engines concurrently.
"""
import numpy as np

S = 32768
H = 1024
N_CORES = 8
S_SHARD = S // N_CORES          # 4096 rows per core
P = 128                         # partitions
N_BLK = S_SHARD // P            # 32 row-blocks per core
# blocks per dma_start: small at the edges (fast pipeline rampup/drain),
# 2 MiB in the middle (DMA efficiency); covers blocks 0..N_BLK-2. The final
# block is streamed as two H-halves so its DMA lands earlier and its
# mult+reduce chain (the only compute on the critical path after the last
# byte arrives) is half as long; the host adds the two partial sums.
DMA_SCHED = [1, 1] + [2] * 14 + [1]
E_CHUNKS = 4                    # output DMA'd in column chunks as it completes

_cache = {}


def _build():
    from concourse import bacc, mybir, tile

    nc = bacc.Bacc("TRN2", target_bir_lowering=False, debug=False,
                   num_devices=N_CORES)
    enc = nc.dram_tensor("enc", [S_SHARD, H], mybir.dt.float32,
                         kind="ExternalInput")
    v_in = nc.dram_tensor("v_in", [1, H], mybir.dt.float32,
                          kind="ExternalInput")
    e_out = nc.dram_tensor("e_out", [P, N_BLK + 1], mybir.dt.float32,
                           kind="ExternalOutput")

    ECW = N_BLK // E_CHUNKS     # columns per output chunk

    with tile.TileContext(nc) as tc:
        with tc.tile_pool(name="const", bufs=1) as cpool, \
             tc.tile_pool(name="psum", bufs=1, space="PSUM") as qpool, \
             tc.tile_pool(name="stream", bufs=8) as spool, \
             tc.tile_pool(name="prod", bufs=4) as ppool, \
             tc.tile_pool(name="cpout", bufs=4) as opool:
            # vt = ones[P,1] @ v[1,H] on PE: avoids streaming 0.5 MB of
            # host-replicated v through the shared HBM stack
            v0 = cpool.tile([1, H], mybir.dt.float32)
            nc.gpsimd.dma_start(out=v0[:], in_=v_in.ap())
            ones = cpool.tile([1, P], mybir.dt.float32)
            nc.vector.memset(ones[:], 1.0)
            pv = qpool.tile([P, H], mybir.dt.float32)
            nc.tensor.matmul(out=pv[:, 0:512], lhsT=ones[:],
                             rhs=v0[:, 0:512], start=True, stop=True)
            nc.tensor.matmul(out=pv[:, 512:H], lhsT=ones[:],
                             rhs=v0[:, 512:H], start=True, stop=True)
            vt = cpool.tile([P, H], mybir.dt.float32)
            nc.scalar.copy(out=vt[:, 0:512], in_=pv[:, 0:512])
            nc.scalar.copy(out=vt[:, 512:H], in_=pv[:, 512:H])
            Es = [cpool.tile([P, ECW], mybir.dt.float32, tag=f"E{k}",
                             name=f"E{k}") for k in range(E_CHUNKS)]
            b0 = 0
            for nb in DMA_SCHED:
                t = spool.tile([P, nb, H], mybir.dt.float32, tag=f"t{nb}")
                rows = enc.ap()[b0 * P:(b0 + nb) * P, :]
                nc.sync.dma_start(out=t[:],
                                  in_=rows.rearrange("(i p) h -> p i h", p=P))
                for i in range(nb):
                    b = b0 + i
                    prod = ppool.tile([P, H], mybir.dt.float32, tag="prod")
                    nc.vector.tensor_tensor(out=prod[:], in0=t[:, i, :],
                                            in1=vt[:],
                                            op=mybir.AluOpType.mult)
                    Et, col = Es[b // ECW], b % ECW
                    cp = opool.tile([P, H], mybir.dt.float32, tag="cp")
                    nc.scalar.activation(
                        out=cp[:], in_=prod[:],
                        func=mybir.ActivationFunctionType.Copy,
                        accum_out=Et[:, col:col + 1])
                b0 += nb
            # final block, split into H-halves; partial sums go to the last
            # two output columns (N_BLK-1 and N_BLK), host adds them
            HH = H // 2
            last = (N_BLK - 1) * P
            Ef = cpool.tile([P, 2], mybir.dt.float32)
            th0 = spool.tile([P, HH], mybir.dt.float32, tag="th")
            th1 = spool.tile([P, HH], mybir.dt.float32, tag="th")
            nc.sync.dma_start(out=th0[:], in_=enc.ap()[last:, 0:HH])
            nc.sync.dma_start(out=th1[:], in_=enc.ap()[last:, HH:H])
            ph0 = ppool.tile([P, HH], mybir.dt.float32, tag="ph")
            nc.vector.tensor_tensor(out=ph0[:], in0=th0[:], in1=vt[:, 0:HH],
                                    op=mybir.AluOpType.mult)
            ph1 = ppool.tile([P, HH], mybir.dt.float32, tag="ph")
            nc.vector.tensor_tensor(out=ph1[:], in0=th1[:], in1=vt[:, HH:H],
                                    op=mybir.AluOpType.mult)
            cpf = opool.tile([P, HH], mybir.dt.float32, tag="cpf")
            nc.scalar.activation(out=cpf[:], in_=ph0[:],
                                 func=mybir.ActivationFunctionType.Copy,
                                 accum_out=Ef[:, 0:1])
            nc.vector.tensor_reduce(out=Ef[:, 1:2], in_=ph1[:],
                                    axis=mybir.AxisListType.X,
                                    op=mybir.AluOpType.add)
            for k in range(E_CHUNKS - 1):
                nc.sync.dma_start(out=e_out.ap()[:, k * ECW:(k + 1) * ECW],
                                  in_=Es[k][:])
            # last chunk stops before col N_BLK-1; the final block's two
            # partial sums own cols N_BLK-1 and N_BLK
            nc.sync.dma_start(
                out=e_out.ap()[:, (E_CHUNKS - 1) * ECW:N_BLK - 1],
                in_=Es[E_CHUNKS - 1][:, 0:ECW - 1])
            nc.sync.dma_start(out=e_out.ap()[:, N_BLK - 1:N_BLK + 1],
                              in_=Ef[:])
    nc.compile()
    return nc


def _get_nc():
    if "nc" not in _cache:
        _cache["nc"] = _build()
    return _cache["nc"]


def kernel(hidden, encoder_outputs, W, b):
    from concourse import bass_utils

    nc = _get_nc()
    h = np.asarray(hidden, dtype=np.float32)[0]
    enc = np.ascontiguousarray(np.asarray(encoder_outputs,
                                          dtype=np.float32)[:, 0, :])
    v = (np.asarray(W, dtype=np.float32).T @ h).astype(np.float32)

    in_maps = [{"enc": enc[c * S_SHARD:(c + 1) * S_SHARD],
                "v_in": v[None, :]} for c in range(N_CORES)]
    res = bass_utils.run_bass_kernel_spmd(
        nc, in_maps, core_ids=list(range(N_CORES)),
        trace=_cache.get("trace", False))
    _cache["last_result"] = res

    # e_out is [partition, block] plus an extra column holding the second
    # partial sum of the final block; global row s = core*4096 + block*128 + p.
    shards = []
    for c in range(N_CORES):
        eo = res.results[c]["e_out"]
        eb = eo[:, :N_BLK].copy()
        eb[:, N_BLK - 1] += eo[:, N_BLK]
        shards.append(eb.T.reshape(S_SHARD))
    e = np.concatenate(shards)
    e = e - e.max()
    p = np.exp(e)
    out = (p / p.sum()).astype(np.float32)
    return out[None, None, :]



# revision 4
# speedup vs baseline: 1.4729x; 1.4729x over previous
"""Trainium2 Bass kernel for nn_Attn: softmax(enc @ (W^T h)) over seq_len.

Math: energy = enc @ W^T + b; attn = energy @ h; out = softmax(attn).
Algebraically attn[s] = enc[s,:] . v + (b.h) with v = W^T h, and the (b.h)
term is constant across s so softmax cancels it. The device work is the
memory-bound part: streaming encoder_outputs once, sharded along seq_len
across 8 NeuronCores. The stream is sent as fp16 (host casts; softmax
rel-err stays ~5e-3, well inside tolerance), halving HBM traffic vs f32.

Per 128-row block the dot with v is either
  - TT path: DVE tensor_tensor multiply (2x 16-bit mode, ~0.68us) and the
    row-sum on the Scalar engine (activation Copy + accum_out, ~1.43us), or
  - STT path: a single DVE scalar_tensor_tensor (in0*1.0)*v with
    accum_out = row-sum (~1.29us, one pass, no Scalar work).
20 TT blocks + 12 STT blocks keeps Vector (~29us) and Scalar (~29us)
both busy just above the ~24.5us DMA stream time. The final block is two
half-width STTs so the drain after the last bytes land is short.
"""
import numpy as np

S = 32768
H = 1024
N_CORES = 8
S_SHARD = S // N_CORES          # 4096 rows per core
P = 128                         # partitions
N_BLK = S_SHARD // P            # 32 row-blocks per core
# blocks per dma_start covering blocks 0..30: small at the edges (fast
# rampup, short drain), 1 MiB in the middle (DMA efficiency). Block 31 is
# streamed as two H-halves afterwards.
DMA_SCHED = [1, 1, 2, 4, 4, 4, 4, 4, 4, 2, 1]
STT_MOD = {2, 5, 7}             # b % 8 in this set -> STT path (blocks < 31)

_cache = {}


def _build():
    from concourse import bacc, mybir, tile

    nc = bacc.Bacc("TRN2", target_bir_lowering=False, debug=False,
                   num_devices=N_CORES)
    enc = nc.dram_tensor("enc", [S_SHARD, H], mybir.dt.float16,
                         kind="ExternalInput")
    v_in = nc.dram_tensor("v_in", [P, H], mybir.dt.float16,
                          kind="ExternalInput")
    # cols 0..30: blocks 0..30; cols 31,32: the two half-sums of block 31
    e_out = nc.dram_tensor("e_out", [P, N_BLK + 1], mybir.dt.float32,
                           kind="ExternalOutput")

    with tile.TileContext(nc) as tc:
        with tc.tile_pool(name="const", bufs=1) as cpool, \
             tc.tile_pool(name="stream", bufs=8) as spool, \
             tc.tile_pool(name="prod", bufs=4) as ppool:
            # v pre-replicated to all 128 partitions on the host (256 KiB);
            # lands on the gpsimd queue while the first stream tiles arrive
            vt = cpool.tile([P, H], mybir.dt.float16)
            nc.gpsimd.dma_start(out=vt[:], in_=v_in.ap())
            E = cpool.tile([P, N_BLK + 1], mybir.dt.float32)
            b0 = 0
            for nb in DMA_SCHED:
                t = spool.tile([P, nb, H], mybir.dt.float16, tag=f"t{nb}")
                rows = enc.ap()[b0 * P:(b0 + nb) * P, :]
                nc.sync.dma_start(out=t[:],
                                  in_=rows.rearrange("(i p) h -> p i h", p=P))
                for i in range(nb):
                    b = b0 + i
                    if b % 8 in STT_MOD:
                        o = ppool.tile([P, H], mybir.dt.float16, tag="so")
                        nc.vector.scalar_tensor_tensor(
                            out=o[:], in0=t[:, i, :], scalar=1.0, in1=vt[:],
                            op0=mybir.AluOpType.mult,
                            op1=mybir.AluOpType.mult,
                            accum_out=E[:, b:b + 1])
                    else:
                        prod = ppool.tile([P, H], mybir.dt.float16,
                                          tag="prod")
                        nc.vector.tensor_tensor(out=prod[:], in0=t[:, i, :],
                                                in1=vt[:],
                                                op=mybir.AluOpType.mult)
                        cp = ppool.tile([P, H], mybir.dt.float16, tag="cp")
                        nc.scalar.activation(
                            out=cp[:], in_=prod[:],
                            func=mybir.ActivationFunctionType.Copy,
                            accum_out=E[:, b:b + 1])
                    if b in (7, 15, 23):
                        k = b // 8
                        nc.sync.dma_start(
                            out=e_out.ap()[:, k * 8:(k + 1) * 8],
                            in_=E[:, k * 8:(k + 1) * 8])
                b0 += nb
            # final block as two half-width STTs: after its (late) bytes
            # land, only ~0.75us of work remains on the critical path
            HH = H // 2
            last = (N_BLK - 1) * P
            th0 = spool.tile([P, HH], mybir.dt.float16, tag="th")
            th1 = spool.tile([P, HH], mybir.dt.float16, tag="th")
            nc.sync.dma_start(out=th0[:], in_=enc.ap()[last:, 0:HH])
            nc.sync.dma_start(out=th1[:], in_=enc.ap()[last:, HH:H])
            oh0 = ppool.tile([P, HH], mybir.dt.float16, tag="oh")
            nc.vector.scalar_tensor_tensor(
                out=oh0[:], in0=th0[:], scalar=1.0, in1=vt[:, 0:HH],
                op0=mybir.AluOpType.mult, op1=mybir.AluOpType.mult,
                accum_out=E[:, N_BLK - 1:N_BLK])
            oh1 = ppool.tile([P, HH], mybir.dt.float16, tag="oh")
            nc.vector.scalar_tensor_tensor(
                out=oh1[:], in0=th1[:], scalar=1.0, in1=vt[:, HH:H],
                op0=mybir.AluOpType.mult, op1=mybir.AluOpType.mult,
                accum_out=E[:, N_BLK:N_BLK + 1])
            nc.sync.dma_start(out=e_out.ap()[:, 24:N_BLK + 1],
                              in_=E[:, 24:N_BLK + 1])
    nc.compile()
    return nc


def _get_nc():
    if "nc" not in _cache:
        _cache["nc"] = _build()
    return _cache["nc"]


def kernel(hidden, encoder_outputs, W, b):
    from concourse import bass_utils

    nc = _get_nc()
    h = np.asarray(hidden, dtype=np.float32)[0]
    enc = np.ascontiguousarray(
        np.asarray(encoder_outputs, dtype=np.float32)[:, 0, :]
    ).astype(np.float16)
    v = (np.asarray(W, dtype=np.float32).T @ h).astype(np.float16)
    vrep = np.ascontiguousarray(np.broadcast_to(v, (P, H)))

    in_maps = [{"enc": enc[c * S_SHARD:(c + 1) * S_SHARD],
                "v_in": vrep} for c in range(N_CORES)]
    res = bass_utils.run_bass_kernel_spmd(
        nc, in_maps, core_ids=list(range(N_CORES)),
        trace=_cache.get("trace", False))
    _cache["last_result"] = res

    # e_out is [partition, block] plus an extra column: cols 31,32 hold the
    # two half-sums of block 31. Global row s = core*4096 + block*128 + p.
    shards = []
    for c in range(N_CORES):
        eo = res.results[c]["e_out"]
        eb = eo[:, :N_BLK].copy()
        eb[:, N_BLK - 1] += eo[:, N_BLK]
        shards.append(eb.T.reshape(S_SHARD))
    e = np.concatenate(shards)
    e = e - e.max()
    p = np.exp(e)
    out = (p / p.sum()).astype(np.float32)
    return out[None, None, :]


# revision 6
# speedup vs baseline: 1.6000x; 1.0863x over previous
"""Trainium2 Bass kernel for nn_Attn: softmax(enc @ (W^T h)) over seq_len.

Math: energy = enc @ W^T + b; attn = energy @ h; out = softmax(attn).
Algebraically attn[s] = enc[s,:] . v + (b.h) with v = W^T h, and the (b.h)
term is constant across s so softmax cancels it. The device work is the
memory-bound part: streaming encoder_outputs once, sharded along seq_len
across 8 NeuronCores. The stream is sent as fp16 (host casts; softmax
rel-err stays ~5e-3, well inside tolerance), halving HBM traffic vs f32.

Each DMA group loads n row-blocks with n CONSECUTIVE rows per partition
("(p j) h -> p j h"), making the per-partition DMA line n*2KiB contiguous
(fp16 rows are only 2 KiB; single-row-per-partition layout measured only
~269 GB/s vs ~342 GB/s for 4 KiB lines). The dot with v is row-wise, so
the row->partition scramble is undone by the host when reassembling.

Per 128-row block the dot with v is either
  - TT path: DVE tensor_tensor multiply (2x 16-bit mode, ~0.68us) and the
    row-sum on the Scalar engine (activation Copy + accum_out, ~1.43us), or
  - STT path: a single DVE scalar_tensor_tensor (in0*1.0)*v with
    accum_out = row-sum (~1.29us, one pass, no Scalar work).
20 TT blocks + 12 STT blocks keeps Vector (~29us) and Scalar (~29us)
both busy just above the ~24.5us DMA stream time. The final block is two
half-width STTs so the drain after the last bytes land is short.
"""
import numpy as np

S = 32768
H = 1024
N_CORES = 8
S_SHARD = S // N_CORES          # 4096 rows per core
P = 128                         # partitions
N_BLK = S_SHARD // P            # 32 row-blocks per core
# row-blocks per dma_start: small at the edges (fast rampup, short drain),
# 1 MiB / 8 KiB-lines in the middle (DMA efficiency). The last block is
# streamed as two H-halves afterwards.
GROUPS = [1, 1, 2, 4, 4, 4, 4, 4, 4, 2, 1]
STT_MOD = {2, 5, 7}             # b % 8 in this set -> STT path (blocks < 31)

_cache = {}


def _build():
    from concourse import bacc, mybir, tile

    nc = bacc.Bacc("TRN2", target_bir_lowering=False, debug=False,
                   num_devices=N_CORES)
    enc = nc.dram_tensor("enc", [S_SHARD, H], mybir.dt.float16,
                         kind="ExternalInput")
    v_in = nc.dram_tensor("v_in", [P, H], mybir.dt.float16,
                          kind="ExternalInput")
    # cols 0..30: blocks 0..30; cols 31,32: the two half-sums of block 31
    e_out = nc.dram_tensor("e_out", [P, N_BLK + 1], mybir.dt.float32,
                           kind="ExternalOutput")

    with tile.TileContext(nc) as tc:
        with tc.tile_pool(name="const", bufs=1) as cpool, \
             tc.tile_pool(name="stream", bufs=6) as spool, \
             tc.tile_pool(name="prod", bufs=4) as ppool:
            # v pre-replicated to all 128 partitions on the host (256 KiB);
            # issued from the Scalar engine's queue so it isn't stuck
            # behind another engine's framework preamble (the gpsimd queue
            # holds a ~3.4us DRAIN that delayed compute start by ~4us)
            vt = cpool.tile([P, H], mybir.dt.float16)
            nc.scalar.dma_start(out=vt[:], in_=v_in.ap())
            E = cpool.tile([P, N_BLK + 1], mybir.dt.float32)
            b0 = 0
            for n in GROUPS:
                t = spool.tile([P, n, H], mybir.dt.float16, tag=f"t{n}")
                rows = enc.ap()[b0 * P:(b0 + n) * P, :]
                # partition p holds rows b0*128 + n*p + j, j<n: n*2KiB
                # contiguous per partition line
                nc.sync.dma_start(out=t[:],
                                  in_=rows.rearrange("(p j) h -> p j h", j=n))
                for j in range(n):
                    b = b0 + j
                    if b % 8 in STT_MOD:
                        o = ppool.tile([P, H], mybir.dt.float16, tag="so")
                        nc.vector.scalar_tensor_tensor(
                            out=o[:], in0=t[:, j, :], scalar=1.0, in1=vt[:],
                            op0=mybir.AluOpType.mult,
                            op1=mybir.AluOpType.mult,
                            accum_out=E[:, b:b + 1])
                    else:
                        prod = ppool.tile([P, H], mybir.dt.float16,
                                          tag="prod")
                        nc.vector.tensor_tensor(out=prod[:], in0=t[:, j, :],
                                                in1=vt[:],
                                                op=mybir.AluOpType.mult)
                        cp = ppool.tile([P, H], mybir.dt.float16, tag="cp")
                        nc.scalar.activation(
                            out=cp[:], in_=prod[:],
                            func=mybir.ActivationFunctionType.Copy,
                            accum_out=E[:, b:b + 1])
                    if b in (7, 15, 23):
                        k = b // 8
                        nc.sync.dma_start(
                            out=e_out.ap()[:, k * 8:(k + 1) * 8],
                            in_=E[:, k * 8:(k + 1) * 8])
                b0 += n
            # final block as two half-width STTs: after its (late) bytes
            # land, only ~0.75us of work remains on the critical path
            HH = H // 2
            last = (N_BLK - 1) * P
            th0 = spool.tile([P, HH], mybir.dt.float16, tag="th")
            th1 = spool.tile([P, HH], mybir.dt.float16, tag="th")
            nc.sync.dma_start(out=th0[:], in_=enc.ap()[last:, 0:HH])
            nc.sync.dma_start(out=th1[:], in_=enc.ap()[last:, HH:H])
            oh0 = ppool.tile([P, HH], mybir.dt.float16, tag="oh")
            nc.vector.scalar_tensor_tensor(
                out=oh0[:], in0=th0[:], scalar=1.0, in1=vt[:, 0:HH],
                op0=mybir.AluOpType.mult, op1=mybir.AluOpType.mult,
                accum_out=E[:, N_BLK - 1:N_BLK])
            oh1 = ppool.tile([P, HH], mybir.dt.float16, tag="oh")
            nc.vector.scalar_tensor_tensor(
                out=oh1[:], in0=th1[:], scalar=1.0, in1=vt[:, HH:H],
                op0=mybir.AluOpType.mult, op1=mybir.AluOpType.mult,
                accum_out=E[:, N_BLK:N_BLK + 1])
            nc.sync.dma_start(out=e_out.ap()[:, 24:N_BLK + 1],
                              in_=E[:, 24:N_BLK + 1])
    nc.compile()
    return nc


def _get_nc():
    if "nc" not in _cache:
        _cache["nc"] = _build()
    return _cache["nc"]


def kernel(hidden, encoder_outputs, W, b):
    from concourse import bass_utils

    nc = _get_nc()
    h = np.asarray(hidden, dtype=np.float32)[0]
    enc = np.ascontiguousarray(
        np.asarray(encoder_outputs, dtype=np.float32)[:, 0, :]
    ).astype(np.float16)
    v = (np.asarray(W, dtype=np.float32).T @ h).astype(np.float16)
    vrep = np.ascontiguousarray(np.broadcast_to(v, (P, H)))

    in_maps = [{"enc": enc[c * S_SHARD:(c + 1) * S_SHARD],
                "v_in": vrep} for c in range(N_CORES)]
    res = bass_utils.run_bass_kernel_spmd(
        nc, in_maps, core_ids=list(range(N_CORES)),
        trace=_cache.get("trace", False))
    _cache["last_result"] = res

    # e_out column b holds, at partition p, the energy of global row
    # core*4096 + 128*B_g + n_g*p + (b - B_g) for the group g containing
    # block b; within a group, eo[:, B:B+n].reshape(-1) is row order.
    # Cols 31,32 are the two half-sums of block 31 (host adds them).
    shards = []
    for c in range(N_CORES):
        eo = res.results[c]["e_out"]
        e_shard = np.empty(S_SHARD, np.float32)
        B = 0
        for n in GROUPS:
            e_shard[128 * B:128 * (B + n)] = eo[:, B:B + n].reshape(-1)
            B += n
        e_shard[128 * (N_BLK - 1):] = eo[:, N_BLK - 1] + eo[:, N_BLK]
        shards.append(e_shard)
    e = np.concatenate(shards)
    e = e - e.max()
    p = np.exp(e)
    out = (p / p.sum()).astype(np.float32)
    return out[None, None, :]
